# revision 10
# baseline (speedup 1.0000x reference)
"""Trainium2 Bass kernel for an AttnBlock (GroupNorm -> QKV 1x1 conv ->
spatial self-attention -> output projection -> residual).

Full-input contract: kernel(**inputs) takes the unsharded numpy inputs and
returns the full (4, 512, 64, 64) float32 output.

Sharding: 8 cores = 4 batches x 2 query-halves. Each core group-norms its
batch, computes Q for its 2048 queries and K/V for all 4096 keys, runs
attention with keys-on-partitions (S^T) layout, and writes its query-half of
the output. The per-core x input is column-rotated on the host so that each
core's own queries are always columns [0, 2048) — this keeps the SPMD
program identical across cores.

Numerics: matmuls in bf16 with fp32 PSUM accumulation; statistics, softmax
denominators and the final combine in fp32. Softmax skips the max-
subtraction (scores are ~N(0,1) after the c^-0.5 scale, so exp never
overflows); denominators come from an all-ones matmul accumulated on the
TensorEngine and are divided out after the output projection.
"""

from contextlib import ExitStack

import numpy as np

import concourse.mybir as mybir
import concourse.tile as tile
from concourse import bacc
from concourse.bass_utils import run_bass_kernel_spmd

# Problem geometry (hardcoded; the grading harness stages only kernel.py).
B = 4
C = 512
HW = 64
N = HW * HW          # 4096 keys per batch
NQ = N // 2          # 2048 queries per core
GROUPS = 32
GSIZE = C // GROUPS  # 16 channels per group
EPS = 1e-6

P = 128
CT = C // P          # 4 channel chunks
JT = N // P          # 32 key chunks of 128
NI = 512             # free-dim tile (queries / keys / channels)
IC = NQ // NI        # 4 query chunks per core
KVN = N // NI        # 8 key n-tiles for the K projection

F32 = mybir.dt.float32
BF16 = mybir.dt.bfloat16

PARAM_NAMES = ("bq", "bk", "bv", "bp", "gn_scale", "gn_bias")
WEIGHT_NAMES = ("wq", "wk", "wv", "wp")

_BUILD_CACHE = {}


def _emit(ctx, nc, tc, x_d, w_d, p_d, out_d):
    AF = mybir.ActivationFunctionType
    ALU = mybir.AluOpType

    consts = ctx.enter_context(tc.tile_pool(name="consts", bufs=1))
    small = ctx.enter_context(tc.tile_pool(name="small", bufs=4))
    stage = ctx.enter_context(tc.tile_pool(name="stage", bufs=2))
    big = ctx.enter_context(tc.tile_pool(name="big", bufs=3))
    qpool = ctx.enter_context(tc.tile_pool(name="qpool", bufs=1))
    wtpool = ctx.enter_context(tc.tile_pool(name="wtpool", bufs=1))
    epool = ctx.enter_context(tc.tile_pool(name="epool", bufs=3))
    attn_pool = ctx.enter_context(tc.tile_pool(name="attn_pool", bufs=2))
    outs_pool = ctx.enter_context(tc.tile_pool(name="outs_pool", bufs=3))
    mm_ps = ctx.enter_context(tc.tile_pool(name="mm_ps", bufs=2, space="PSUM"))
    acc_ps = ctx.enter_context(tc.tile_pool(name="acc_ps", bufs=5, space="PSUM"))

    # ---- constants -------------------------------------------------------
    # Per-channel params as (128, CT): column cc = channels [cc*128, cc*128+128).
    par = {}
    for name in PARAM_NAMES:
        t = consts.tile([P, CT], F32, tag=f"par_{name}", name=f"par_{name}")
        nc.gpsimd.dma_start(out=t, in_=p_d[name][:].rearrange("(t p) -> p t", p=P))
        par[name] = t
    # bv broadcast across partitions (free dim = c_out) for the V^T eviction.
    bv_bc = consts.tile([P, C], F32, tag="bv_bc")
    nc.gpsimd.dma_start(out=bv_bc, in_=p_d["bv"][:].partition_broadcast(P))

    ident = consts.tile([P, P], F32, tag="ident")
    nc.gpsimd.memset(ident, 0.0)
    nc.gpsimd.affine_select(
        out=ident, in_=ident, compare_op=ALU.not_equal, fill=1.0,
        base=0, pattern=[[-1, P]], channel_multiplier=1,
    )
    ones128 = consts.tile([P, P], BF16, tag="ones128")
    nc.vector.memset(ones128, 1.0)

    # Group-reduction matrices. G: (128, 8) with G[p, g] = 1/GSIZE iff
    # p // GSIZE == g. GE: (8, 128) with GE[g, p] = 1 iff p // GSIZE == g.
    GPC = P // GSIZE  # 8 groups per 128-channel chunk
    gmat = consts.tile([P, GPC], F32, tag="gmat")
    nc.gpsimd.memset(gmat, 1.0 / GSIZE)
    nc.gpsimd.affine_select(
        out=gmat, in_=gmat, compare_op=ALU.is_ge, fill=0.0,
        base=0, pattern=[[-GSIZE, GPC]], channel_multiplier=1,
    )
    # p - GSIZE*g <= GSIZE-1  rewritten as  (GSIZE-1) - p + GSIZE*g >= 0
    nc.gpsimd.affine_select(
        out=gmat, in_=gmat, compare_op=ALU.is_ge, fill=0.0,
        base=GSIZE - 1, pattern=[[GSIZE, GPC]], channel_multiplier=-1,
    )
    gexp = consts.tile([GPC, P], F32, tag="gexp")
    nc.gpsimd.memset(gexp, 1.0)
    nc.gpsimd.affine_select(
        out=gexp, in_=gexp, compare_op=ALU.is_ge, fill=0.0,
        base=0, pattern=[[1, P]], channel_multiplier=-GSIZE,
    )
    nc.gpsimd.affine_select(
        out=gexp, in_=gexp, compare_op=ALU.is_ge, fill=0.0,
        base=GSIZE - 1, pattern=[[-1, P]], channel_multiplier=GSIZE,
    )
    eps8 = consts.tile([GPC, 1], F32, tag="eps8")
    nc.vector.memset(eps8, EPS)

    # ---- weights: load f32, PE-transpose to (c_in, c_out), cast bf16 -----
    wt = {}
    for wname in WEIGHT_NAMES:
        wt[wname] = wtpool.tile([P, CT, C], BF16, tag=f"wt_{wname}",
                                name=f"wt_{wname}")
    for wname in WEIGHT_NAMES:
        for oc in range(CT):
            ws = stage.tile([P, C], F32, tag="wstage")
            nc.sync.dma_start(out=ws, in_=w_d[wname][oc * P:(oc + 1) * P, :])
            for icc in range(CT):
                tp = mm_ps.tile([P, P], F32, tag="mm")
                nc.tensor.transpose(out=tp, in_=ws[:, icc * P:(icc + 1) * P],
                                    identity=ident)
                nc.vector.tensor_copy(out=wt[wname][:, icc, oc * P:(oc + 1) * P],
                                      in_=tp)

    # ---- x load + GroupNorm stats + normalize (to bf16 h) ----------------
    xbf = big.tile([P, CT, N], BF16, tag="big")
    h = big.tile([P, CT, N], BF16, tag="big")
    for cc in range(CT):
        stats = small.tile([P, 8, 6], F32, tag="gn_stats")
        for half in range(2):
            xs = stage.tile([P, N // 2], F32, tag="xstage")
            nc.sync.dma_start(
                out=xs, in_=x_d[cc * P:(cc + 1) * P, half * (N // 2):(half + 1) * (N // 2)])
            for sg in range(4):
                nc.vector.bn_stats(out=stats[:, half * 4 + sg, :],
                                   in_=xs[:, sg * NI:(sg + 1) * NI])
            nc.vector.tensor_copy(
                out=xbf[:, cc, half * (N // 2):(half + 1) * (N // 2)], in_=xs)
        mv = small.tile([P, 2], F32, tag="gn_mv")
        nc.vector.bn_aggr(out=mv, in_=stats)
        # stat2 = [mean_c, E[x^2]_c]
        stat2 = small.tile([P, 2], F32, tag="gn_stat2")
        nc.vector.tensor_copy(out=stat2[:, 0:1], in_=mv[:, 0:1])
        sq = small.tile([P, 1], F32, tag="gn_sq")
        nc.vector.tensor_mul(out=sq, in0=mv[:, 0:1], in1=mv[:, 0:1])
        nc.vector.tensor_add(out=stat2[:, 1:2], in0=mv[:, 1:2], in1=sq)
        # group-combine on PE: (8, 2) = G^T @ stat2 (entries are means over 16ch)
        g_ps = mm_ps.tile([GPC, 2], F32, tag="mm")
        nc.tensor.matmul(g_ps, lhsT=gmat, rhs=stat2, start=True, stop=True)
        g_sb = small.tile([GPC, 2], F32, tag="gn_gsb")
        nc.vector.tensor_copy(out=g_sb, in_=g_ps)
        # grp = [mean_g, rstd_g]
        grp = small.tile([GPC, 2], F32, tag="gn_grp")
        nc.vector.tensor_copy(out=grp[:, 0:1], in_=g_sb[:, 0:1])
        sq2 = small.tile([GPC, 1], F32, tag="gn_sq2")
        nc.vector.tensor_mul(out=sq2, in0=g_sb[:, 0:1], in1=g_sb[:, 0:1])
        var = small.tile([GPC, 1], F32, tag="gn_var")
        nc.vector.tensor_sub(out=var, in0=g_sb[:, 1:2], in1=sq2)
        sd = small.tile([GPC, 1], F32, tag="gn_sd")
        nc.scalar.activation(out=sd, in_=var, func=AF.Sqrt, bias=eps8, scale=1.0)
        nc.vector.reciprocal(out=grp[:, 1:2], in_=sd)
        # expand back to per-channel via PE: (128, 2) = GE^T @ grp
        e_ps = mm_ps.tile([P, 2], F32, tag="mm")
        nc.tensor.matmul(e_ps, lhsT=gexp, rhs=grp, start=True, stop=True)
        e_sb = small.tile([P, 2], F32, tag="gn_esb")
        nc.vector.tensor_copy(out=e_sb, in_=e_ps)
        # a_c = gn_scale * rstd ; b_c = gn_bias - mean * a_c
        a_c = small.tile([P, 1], F32, tag="gn_a")
        nc.vector.tensor_mul(out=a_c, in0=par["gn_scale"][:, cc:cc + 1],
                             in1=e_sb[:, 1:2])
        tmp = small.tile([P, 1], F32, tag="gn_tmp")
        nc.vector.tensor_mul(out=tmp, in0=e_sb[:, 0:1], in1=a_c)
        b_c = small.tile([P, 1], F32, tag="gn_b")
        nc.vector.tensor_sub(out=b_c, in0=par["gn_bias"][:, cc:cc + 1], in1=tmp)
        # h = a_c * x + b_c   (bf16 output)
        nc.vector.tensor_scalar(
            out=h[:, cc, :], in0=xbf[:, cc, :], scalar1=a_c, scalar2=b_c,
            op0=ALU.mult, op1=ALU.add)

    # ---- projections -----------------------------------------------------
    # Q: (c_out, i) for our 2048 queries.
    q_sb = qpool.tile([P, CT, NQ], BF16, tag="q")
    for oc in range(CT):
        for icq in range(IC):
            ps = mm_ps.tile([P, NI], F32, tag="mm")
            for icc in range(CT):
                nc.tensor.matmul(
                    ps, lhsT=wt["wq"][:, icc, oc * P:(oc + 1) * P],
                    rhs=h[:, icc, icq * NI:(icq + 1) * NI],
                    start=(icc == 0), stop=(icc == CT - 1))
            nc.vector.tensor_scalar_add(
                out=q_sb[:, oc, icq * NI:(icq + 1) * NI], in0=ps,
                scalar1=par["bq"][:, oc:oc + 1])
    # K: (c_out, j) over all 4096 keys.
    k_sb = big.tile([P, CT, N], BF16, tag="big")
    for oc in range(CT):
        for jn in range(KVN):
            ps = mm_ps.tile([P, NI], F32, tag="mm")
            for icc in range(CT):
                nc.tensor.matmul(
                    ps, lhsT=wt["wk"][:, icc, oc * P:(oc + 1) * P],
                    rhs=h[:, icc, jn * NI:(jn + 1) * NI],
                    start=(icc == 0), stop=(icc == CT - 1))
            nc.vector.tensor_scalar_add(
                out=k_sb[:, oc, jn * NI:(jn + 1) * NI], in0=ps,
                scalar1=par["bk"][:, oc:oc + 1])
    # V^T: (j, c_out) computed directly (lhsT = h key-chunk, rhs = wv^T).
    vt_sb = big.tile([P, JT, C], BF16, tag="big")
    for jc in range(JT):
        ps = mm_ps.tile([P, C], F32, tag="mm")
        for icc in range(CT):
            nc.tensor.matmul(
                ps, lhsT=h[:, icc, jc * P:(jc + 1) * P], rhs=wt["wv"][:, icc, :],
                start=(icc == 0), stop=(icc == CT - 1))
        nc.vector.tensor_add(out=vt_sb[:, jc, :], in0=ps, in1=bv_bc)

    # ---- attention + output projection + residual ------------------------
    inv_sqrt_c = float(C) ** -0.5
    for icq in range(IC):
        att_ps = [acc_ps.tile([P, NI], F32, tag="acc", name=f"att_ps_{icq}_{ct}")
                  for ct in range(CT)]
        den_ps = acc_ps.tile([P, NI], F32, tag="acc")
        for jc in range(JT):
            s_ps = mm_ps.tile([P, NI], F32, tag="mm")
            for icc in range(CT):
                nc.tensor.matmul(
                    s_ps, lhsT=k_sb[:, icc, jc * P:(jc + 1) * P],
                    rhs=q_sb[:, icc, icq * NI:(icq + 1) * NI],
                    start=(icc == 0), stop=(icc == CT - 1))
            e = epool.tile([P, NI], BF16, tag="e")
            nc.scalar.activation(out=e, in_=s_ps, func=AF.Exp, scale=inv_sqrt_c)
            for ct in range(CT):
                nc.tensor.matmul(
                    att_ps[ct], lhsT=vt_sb[:, jc, ct * P:(ct + 1) * P], rhs=e,
                    start=(jc == 0), stop=(jc == JT - 1))
            nc.tensor.matmul(
                den_ps, lhsT=ones128, rhs=e,
                start=(jc == 0), stop=(jc == JT - 1))
        att_sb = attn_pool.tile([P, CT, NI], BF16, tag="attn")
        for ct in range(CT):
            nc.vector.tensor_copy(out=att_sb[:, ct, :], in_=att_ps[ct])
        rec = outs_pool.tile([P, NI], F32, tag="rec")
        nc.vector.reciprocal(out=rec, in_=den_ps)
        for dc in range(CT):
            pp = mm_ps.tile([P, NI], F32, tag="mm")
            for ct in range(CT):
                nc.tensor.matmul(
                    pp, lhsT=wt["wp"][:, ct, dc * P:(dc + 1) * P],
                    rhs=att_sb[:, ct, :],
                    start=(ct == 0), stop=(ct == CT - 1))
            xr = outs_pool.tile([P, NI], F32, tag="xres")
            nc.sync.dma_start(
                out=xr, in_=x_d[dc * P:(dc + 1) * P, icq * NI:(icq + 1) * NI])
            ob = outs_pool.tile([P, NI], F32, tag="ob")
            nc.vector.tensor_mul(out=ob, in0=pp, in1=rec)
            nc.vector.tensor_scalar_add(out=ob, in0=ob,
                                        scalar1=par["bp"][:, dc:dc + 1])
            nc.vector.tensor_add(out=ob, in0=ob, in1=xr)
            nc.sync.dma_start(
                out=out_d[dc * P:(dc + 1) * P, icq * NI:(icq + 1) * NI], in_=ob)


def _build():
    nc = bacc.Bacc()
    x_d = nc.declare_dram_parameter("x", [C, N], F32, isOutput=False)
    w_d = {w: nc.declare_dram_parameter(w, [C, C], F32, isOutput=False)
           for w in WEIGHT_NAMES}
    p_d = {p: nc.declare_dram_parameter(p, [C], F32, isOutput=False)
           for p in PARAM_NAMES}
    out_d = nc.declare_dram_parameter("out", [C, NQ], F32, isOutput=True)
    with tile.TileContext(nc) as tc, ExitStack() as ctx:
        _emit(ctx, nc, tc, x_d, w_d, p_d, out_d)
    nc.finalize()
    return nc


def _get_nc():
    if "nc" not in _BUILD_CACHE:
        _BUILD_CACHE["nc"] = _build()
    return _BUILD_CACHE["nc"]


def _make_in_maps(x, gn_scale, gn_bias, wq, bq, wk, bk, wv, bv, wp, bp):
    xf = np.ascontiguousarray(np.asarray(x, dtype=np.float32).reshape(B, C, N))
    shared = {
        "wq": np.ascontiguousarray(np.asarray(wq, np.float32)),
        "wk": np.ascontiguousarray(np.asarray(wk, np.float32)),
        "wv": np.ascontiguousarray(np.asarray(wv, np.float32)),
        "wp": np.ascontiguousarray(np.asarray(wp, np.float32)),
        "bq": np.ascontiguousarray(np.asarray(bq, np.float32)),
        "bk": np.ascontiguousarray(np.asarray(bk, np.float32)),
        "bv": np.ascontiguousarray(np.asarray(bv, np.float32)),
        "bp": np.ascontiguousarray(np.asarray(bp, np.float32)),
        "gn_scale": np.ascontiguousarray(np.asarray(gn_scale, np.float32)),
        "gn_bias": np.ascontiguousarray(np.asarray(gn_bias, np.float32)),
    }
    in_maps = []
    for core in range(8):
        bi, qh = core // 2, core % 2
        xb = xf[bi]
        if qh == 0:
            xc = xb
        else:
            xc = np.ascontiguousarray(
                np.concatenate([xb[:, NQ:], xb[:, :NQ]], axis=1))
        in_maps.append({"x": xc, **shared})
    return in_maps


def _gather(results):
    out = np.empty((B, C, N), np.float32)
    for core in range(8):
        bi, qh = core // 2, core % 2
        out[bi, :, qh * NQ:(qh + 1) * NQ] = results[core]["out"]
    return out.reshape(B, C, HW, HW)


def kernel(x, gn_scale, gn_bias, wq, bq, wk, bk, wv, bv, wp, bp):
    nc = _get_nc()
    in_maps = _make_in_maps(x, gn_scale, gn_bias, wq, bq, wk, bk, wv, bv,
                            wp, bp)
    res = run_bass_kernel_spmd(nc, in_maps, core_ids=list(range(8)))
    return _gather(res.results)
